# revision 10
# baseline (speedup 1.0000x reference)
"""Trainium2 Bass kernel for BasicDGCNN forward (4x8192x5 points, k=20).

Sharding: 8 cores = 4 samples x 2 query-halves. Each core computes kNN +
EdgeConv for its 4096 query rows against all 8192 candidates of its sample.
Pairwise AllGather exchanges layer features between the two half-cores.

Key algebra (all monotone-safe):
  dist ordering:  argsort(dist_ij) == argsort(-(2<xi,xj> - xx_j))
  EdgeConv:       y = max_j LReLU(s*(Wc@xi + Wn@(xj-xi) - m) + b)
                    = LReLU(s*a_i + max_j s*u_j + c2)   [s>0, LReLU monotone]
  with a = (Wc-Wn)@x, u = Wn@x, s = g*rsqrt(v+eps), c2 = b - s*m.
"""

import numpy as np

import concourse.bass as bass
import concourse.mybir as mybir
from concourse import bacc
from concourse.tile import TileContext
from concourse.bass_utils import run_bass_kernel_spmd
from concourse.masks import make_identity

F32 = mybir.dt.float32
U16 = mybir.dt.uint16
I16 = mybir.dt.int16
AX = mybir.AxisListType
OP = mybir.AluOpType
AF = mybir.ActivationFunctionType

B = 4
N_FULL = 8192
D_IN = 5
KNN = 20
EPS = 1e-5
MAXV = 64
NEG = -3.0e38

# (C_in, O_out, O_padded_to_256B)
LAYERS = [(5, 32, 64), (32, 64, 64), (64, 128, 128)]


def build_kernel(n_pts, n_cores):
    """Build the SPMD Bass program. Each core handles n_pts//2 query rows."""
    half = n_pts // 2
    qblocks = half // 128
    cblocks = n_pts // 128
    chunks = n_pts // 512
    pairs = [[2 * i, 2 * i + 1] for i in range(n_cores // 2)]

    nc = bacc.Bacc(None, target_bir_lowering=False, debug=False,
                   num_devices=n_cores)

    # ---- I/O ----
    xt0 = nc.dram_tensor("xt0", [D_IN + 1, n_pts], F32, kind="ExternalInput")
    al1 = nc.dram_tensor("al1", [D_IN + 1, half], F32, kind="ExternalInput")
    wu_d, wa_d = [], []
    for li, (C, O, _Op) in enumerate(LAYERS):
        wu_d.append(nc.dram_tensor(f"wu{li}", [C, O], F32, kind="ExternalInput"))
        wa_d.append(nc.dram_tensor(f"wa{li}", [C + 1, O], F32, kind="ExternalInput"))
    wga_d = nc.dram_tensor("wga", [128, 256], F32, kind="ExternalInput")
    wgb_d = nc.dram_tensor("wgb", [97, 256], F32, kind="ExternalInput")
    wv1_d = nc.dram_tensor("wv1", [128, 2, 512], F32, kind="ExternalInput")
    bv1_d = nc.dram_tensor("bv1", [128, 4], F32, kind="ExternalInput")
    wv2_d = nc.dram_tensor("wv2", [128, 4, 192], F32, kind="ExternalInput")
    bv2_d = nc.dram_tensor("bv2", [96, 2], F32, kind="ExternalInput")
    wq1_d = nc.dram_tensor("wq1", [128, 2, 64], F32, kind="ExternalInput")
    bq1_d = nc.dram_tensor("bq1", [64, 1], F32, kind="ExternalInput")
    wq2_d = nc.dram_tensor("wq2", [64, 1], F32, kind="ExternalInput")
    bq2_d = nc.dram_tensor("bq2", [1, 1], F32, kind="ExternalInput")

    xc_out = nc.dram_tensor("xc", [half, 224], F32, kind="ExternalOutput")
    gf_out = nc.dram_tensor("gf", [128, 2], F32, kind="ExternalOutput")
    vc_out = nc.dram_tensor("vc", [96, 2], F32, kind="ExternalOutput")
    nv_out = nc.dram_tensor("nv", [1, 1], F32, kind="ExternalOutput")

    with TileContext(nc) as tc:
        with (
            tc.tile_pool(name="xt", bufs=1) as xt_pool,
            tc.tile_pool(name="al", bufs=1) as al_pool,
            tc.tile_pool(name="sco", bufs=2) as s_pool,
            tc.tile_pool(name="xct", bufs=1) as xct_pool,
            tc.tile_pool(name="gth", bufs=2) as g_pool,
            tc.tile_pool(name="sml", bufs=3) as sm_pool,
            tc.tile_pool(name="cst", bufs=1) as c_pool,
            tc.tile_pool(name="ps", bufs=4, space="PSUM") as psum_s,
            tc.tile_pool(name="pm", bufs=3, space="PSUM") as psum_m,
            tc.tile_pool(name="dram", bufs=1, space="DRAM") as dram,
        ):
            # ---- constants in SBUF ----
            ident = c_pool.tile([128, 128], F32, tag="ident")
            make_identity(nc, ident[:])
            ones_col = c_pool.tile([128, 1], F32, tag="ones")
            nc.gpsimd.memset(ones_col[:], 1.0)

            wu_sb, wa_sb = [], []
            for li, (C, O, _Op) in enumerate(LAYERS):
                wu = c_pool.tile([C, O], F32, tag=f"wu{li}")
                nc.sync.dma_start(wu[:], wu_d[li][:])
                wu_sb.append(wu)
                wa = c_pool.tile([C + 1, O], F32, tag=f"wa{li}")
                nc.sync.dma_start(wa[:], wa_d[li][:])
                wa_sb.append(wa)

            wga = c_pool.tile([128, 256], F32, tag="wga")
            nc.sync.dma_start(wga[:], wga_d[:])
            wgb = c_pool.tile([97, 256], F32, tag="wgb")
            nc.sync.dma_start(wgb[:], wgb_d[:])
            wv1 = c_pool.tile([128, 2, 512], F32, tag="wv1")
            nc.sync.dma_start(wv1[:], wv1_d[:])
            bv1 = c_pool.tile([128, 4], F32, tag="bv1")
            nc.sync.dma_start(bv1[:], bv1_d[:])
            wv2 = c_pool.tile([128, 4, 192], F32, tag="wv2")
            nc.sync.dma_start(wv2[:], wv2_d[:])
            bv2 = c_pool.tile([96, 2], F32, tag="bv2")
            nc.sync.dma_start(bv2[:], bv2_d[:])
            wq1 = c_pool.tile([128, 2, 64], F32, tag="wq1")
            nc.sync.dma_start(wq1[:], wq1_d[:])
            bq1 = c_pool.tile([64, 1], F32, tag="bq1")
            nc.sync.dma_start(bq1[:], bq1_d[:])
            wq2 = c_pool.tile([64, 1], F32, tag="wq2")
            nc.sync.dma_start(wq2[:], wq2_d[:])
            bq2 = c_pool.tile([1, 1], F32, tag="bq2")
            nc.sync.dma_start(bq2[:], bq2_d[:])

            # concat-transposed features (queries of this core, 1x scale)
            # rows: x1 -> a[0:32], x2 -> a[32:96], x3 -> a[96:128]+b[0:96]
            xct_a = xct_pool.tile([128, half], F32, tag="xct_a")
            xct_b = xct_pool.tile([97, half], F32, tag="xct_b")
            nc.gpsimd.memset(xct_b[96:97, :], 1.0)  # ones row for c2g fold

            # ---- layer 0 inputs ----
            XT = xt_pool.tile([D_IN + 1, n_pts], F32, tag="xt")
            nc.sync.dma_start(XT[:], xt0[:])
            AL = al_pool.tile([D_IN + 1, half], F32, tag="al")
            nc.sync.dma_start(AL[:], al1[:])

            col_off = 0
            for li, (C, O, Opad) in enumerate(LAYERS):
                # --- U table: u'_j = s * Wn @ x_j  for all candidates ---
                U = dram.tile([n_pts, Opad], F32, tag="U")
                for cb in range(cblocks):
                    pu = psum_m.tile([128, O], F32, tag="pm")
                    nc.tensor.matmul(
                        out=pu[:], lhsT=XT[0:C, cb * 128:(cb + 1) * 128],
                        rhs=wu_sb[li][0:C, 0:O], start=True, stop=True)
                    us = sm_pool.tile([128, Opad], F32, tag="us")
                    nc.scalar.activation(us[:, 0:O], pu[:], AF.Copy)
                    if Opad > O:
                        nc.gpsimd.memset(us[:, O:Opad], 0.0)
                    nc.sync.dma_start(U[cb * 128:(cb + 1) * 128, :], us[:])

                # --- per query block ---
                for qb in range(qblocks):
                    qs = slice(qb * 128, (qb + 1) * 128)
                    S = s_pool.tile([128, n_pts], F32, tag="S")
                    for ch in range(chunks):
                        ps = psum_s.tile([128, 512], F32, tag="ps")
                        nc.tensor.matmul(
                            out=ps[:], lhsT=AL[0:C + 1, qs],
                            rhs=XT[0:C + 1, ch * 512:(ch + 1) * 512],
                            start=True, stop=True)
                        nc.scalar.activation(
                            S[:, ch * 512:(ch + 1) * 512], ps[:], AF.Copy)

                    maxv = sm_pool.tile([128, 8], F32, tag="maxv")
                    idxu = sm_pool.tile([128, 24], U16, tag="idxu")
                    for r in range(3):
                        nc.vector.max(out=maxv[:], in_=S[:])
                        nc.vector.max_index(
                            out=idxu[:, 8 * r:8 * r + 8], in_max=maxv[:],
                            in_values=S[:])
                        if r < 2:
                            nc.vector.match_replace(
                                out=S[:], in_to_replace=maxv[:],
                                in_values=S[:], imm_value=NEG)

                    # fold [128,20] -> wrapped list widx[j%16, j//16], j=q+128*s
                    # (list lives in 16 partitions, replicated to all 8 Q7
                    # core groups)
                    widx = sm_pool.tile([128, 8 * KNN], U16, tag="widx")
                    wv3 = widx[0:16, :].rearrange("p (s r) -> p s r", r=8)
                    for r in range(8):
                        nc.sync.dma_start(
                            wv3[:, :, r], idxu[16 * r:16 * (r + 1), 1:KNN + 1])
                    for grp in range(1, 8):
                        nc.sync.dma_start(
                            widx[16 * grp:16 * (grp + 1), :], widx[0:16, :])

                    # descriptor scratch fits 1024 descs -> chunk the gather
                    G = g_pool.tile([128, KNN, Opad], F32, tag="G")
                    for s0 in range(0, KNN, 8):
                        sn = min(8, KNN - s0)
                        nc.gpsimd.dma_gather(
                            out_ap=G[:, s0:s0 + sn, :],
                            in_ap=U[:],
                            idxs_ap=widx[:, 8 * s0:8 * (s0 + sn)].bitcast(I16),
                            num_idxs=sn * 128, num_idxs_reg=sn * 128,
                            elem_size=Opad)

                    nmax = sm_pool.tile([128, O], F32, tag="nmax")
                    gv = G[:].rearrange("p t o -> p o t")[:, 0:O, :]
                    nc.vector.tensor_reduce(
                        out=nmax[:], in_=gv, axis=AX.X, op=OP.max)

                    pa = psum_m.tile([128, O], F32, tag="pm")
                    nc.tensor.matmul(
                        out=pa[:], lhsT=AL[0:C + 1, qs],
                        rhs=wa_sb[li][0:C + 1, 0:O], start=True, stop=True)
                    orow = sm_pool.tile([128, O], F32, tag="orow")
                    nc.vector.tensor_add(orow[:], pa[:], nmax[:])
                    nc.vector.scalar_tensor_tensor(
                        out=orow[:], in0=orow[:], scalar=0.2, in1=orow[:],
                        op0=OP.mult, op1=OP.max)

                    nc.sync.dma_start(xc_out[qs, col_off:col_off + O], orow[:])

                    pt = psum_m.tile([128, 128], F32, tag="pm")
                    nc.tensor.transpose(pt[0:O, :], orow[:], ident[:])
                    # copy rows of pt into xcT in 32-partition-aligned chunks
                    for po in range(0, O, 32):
                        ro = col_off + po
                        if ro < 128:
                            dst = xct_a[ro:ro + 32, qs]
                        else:
                            dst = xct_b[ro - 128:ro - 96, qs]
                        nc.scalar.activation(dst, pt[po:po + 32, :], AF.Copy)

                # --- exchange halves, build next XT/AL ---
                if li < 2:
                    C2 = LAYERS[li + 1][0]  # == O
                    bin_ = dram.tile([O, half], F32, tag="bin")
                    nc.sync.dma_start(bin_[:], xct_a[col_off:col_off + O, :])
                    bout = dram.tile([2, O, half], F32, tag="bout")
                    nc.gpsimd.collective_compute(
                        "AllGather", OP.bypass, replica_groups=pairs,
                        ins=[bin_.opt()], outs=[bout.opt()])
                    XT = xt_pool.tile([C2 + 1, n_pts], F32, tag="xt")
                    nc.sync.dma_start(XT[0:C2, 0:half], bout[0])
                    nc.sync.dma_start(XT[0:C2, half:n_pts], bout[1])
                    # double in place: rows hold 2*x^T
                    nc.scalar.activation(XT[0:C2, :], XT[0:C2, :], AF.Copy,
                                         scale=2.0)
                    # row C2 = -xx  (Square(2x) = 4x^2, scale by -1/4)
                    for ch in range(chunks):
                        cs = slice(ch * 512, (ch + 1) * 512)
                        sq = sm_pool.tile([C2, 512], F32, tag="sq")
                        nc.scalar.activation(sq[:], XT[0:C2, cs], AF.Square)
                        px = psum_m.tile([1, 512], F32, tag="pm")
                        nc.tensor.matmul(out=px[:], lhsT=ones_col[0:C2, :],
                                         rhs=sq[:], start=True, stop=True)
                        nc.scalar.activation(XT[C2:C2 + 1, cs], px[:],
                                             AF.Copy, scale=-0.25)
                    AL = al_pool.tile([C2 + 1, half], F32, tag="al")
                    # partition-aligned copies (base%32==0, span<=32 unless
                    # base%64==0)
                    for po in range(0, O, 32):
                        span = min(32, O - po)
                        nc.scalar.activation(
                            AL[po:po + span, :],
                            xct_a[col_off + po:col_off + po + span, :], AF.Copy)
                    nc.gpsimd.memset(AL[C2:C2 + 1, :], 1.0)
                col_off += O

            # ---- conv_global + global max ----
            gacc = c_pool.tile([128, 256], F32, tag="gacc")
            for qb in range(qblocks):
                qs = slice(qb * 128, (qb + 1) * 128)
                pg = psum_m.tile([128, 256], F32, tag="pm")
                nc.tensor.matmul(out=pg[:], lhsT=xct_a[:, qs], rhs=wga[:],
                                 start=True, stop=False)
                nc.tensor.matmul(out=pg[:], lhsT=xct_b[:, qs], rhs=wgb[:],
                                 start=False, stop=True)
                # LReLU without double-PSUM-read: t = 0.2*pg; xg = max(t, pg)
                xgl = sm_pool.tile([128, 256], F32, tag="xgl")
                nc.vector.tensor_scalar_mul(xgl[:], pg[:], 0.2)
                xg = sm_pool.tile([128, 256], F32, tag="xg")
                nc.vector.tensor_max(xg[:], xgl[:], pg[:])
                if qb == 0:
                    nc.vector.tensor_copy(gacc[:], xg[:])
                else:
                    nc.vector.tensor_max(gacc[:], gacc[:], xg[:])

            pt1 = psum_m.tile([128, 128], F32, tag="pm")
            nc.tensor.transpose(pt1[:], gacc[:, 0:128], ident[:])
            pt2 = psum_m.tile([128, 128], F32, tag="pm")
            nc.tensor.transpose(pt2[:], gacc[:, 128:256], ident[:])
            gT = sm_pool.tile([128, 2, 128], F32, tag="gT")
            nc.scalar.activation(gT[:, 0, :], pt1[:], AF.Copy)
            nc.scalar.activation(gT[:, 1, :], pt2[:], AF.Copy)
            gloc = sm_pool.tile([128, 2], F32, tag="gloc")
            nc.vector.tensor_reduce(out=gloc[:], in_=gT[:], axis=AX.X,
                                    op=OP.max)
            gbi = dram.tile([128, 2], F32, tag="gbi")
            nc.sync.dma_start(gbi[:], gloc[:])
            gbo = dram.tile([128, 2], F32, tag="gbo")
            nc.gpsimd.collective_compute(
                "AllReduce", OP.max, replica_groups=pairs,
                ins=[gbi.opt()], outs=[gbo.opt()])
            gf = c_pool.tile([128, 2], F32, tag="gf")
            nc.sync.dma_start(gf[:], gbo[:])
            nc.sync.dma_start(gf_out[:], gf[:])

            # ---- heads ----
            h_sb = sm_pool.tile([128, 4], F32, tag="h")
            for j in range(4):
                ph = psum_m.tile([128, 1], F32, tag="pm")
                for c in range(2):
                    nc.tensor.matmul(
                        out=ph[:], lhsT=wv1[:, c, j * 128:(j + 1) * 128],
                        rhs=gf[:, c:c + 1], start=(c == 0), stop=(c == 1))
                nc.scalar.activation(h_sb[:, j:j + 1], ph[:], AF.Relu,
                                     bias=bv1[:, j:j + 1])
            v_sb = sm_pool.tile([96, 2], F32, tag="v")
            for j in range(2):
                pv = psum_m.tile([96, 1], F32, tag="pm")
                for c in range(4):
                    nc.tensor.matmul(
                        out=pv[:], lhsT=wv2[:, c, j * 96:(j + 1) * 96],
                        rhs=h_sb[:, c:c + 1], start=(c == 0), stop=(c == 3))
                nc.vector.tensor_add(v_sb[:, j:j + 1], pv[:], bv2[:, j:j + 1])
            nc.sync.dma_start(vc_out[:], v_sb[:])

            q_sb = sm_pool.tile([64, 1], F32, tag="q")
            pq = psum_m.tile([64, 1], F32, tag="pm")
            for c in range(2):
                nc.tensor.matmul(out=pq[:], lhsT=wq1[:, c, 0:64],
                                 rhs=gf[:, c:c + 1], start=(c == 0),
                                 stop=(c == 1))
            nc.scalar.activation(q_sb[:], pq[:], AF.Relu, bias=bq1[:])
            pn = psum_m.tile([1, 1], F32, tag="pm")
            nc.tensor.matmul(out=pn[:], lhsT=wq2[:], rhs=q_sb[:],
                             start=True, stop=True)
            nv_sb = sm_pool.tile([1, 1], F32, tag="nvs")
            nc.scalar.activation(nv_sb[:], pn[:], AF.Sigmoid, bias=bq2[:])
            nc.sync.dma_start(nv_out[:], nv_sb[:])

    nc.compile()
    return nc


def prep_inputs(inputs, n_pts, n_cores):
    """Build per-core input maps (numpy, float32)."""
    f = np.float32
    d = {k: np.ascontiguousarray(np.asarray(v, f)) for k, v in inputs.items()}
    half = n_pts // 2
    shared = {}
    for li, (C, O, _Op) in enumerate(LAYERS):
        W = d[f"W{li + 1}"]
        g, b, m, v = d[f"g{li + 1}"], d[f"b{li + 1}"], d[f"m{li + 1}"], d[f"v{li + 1}"]
        s = (g * (1.0 / np.sqrt(v + f(EPS)))).astype(f)
        c2 = (b - s * m).astype(f)
        Wc, Wn = W[:, :C], W[:, C:]
        shared[f"wu{li}"] = np.ascontiguousarray((Wn.T * s[None, :]) * f(0.5))
        shared[f"wa{li}"] = np.ascontiguousarray(
            np.concatenate([(Wc - Wn).T * s[None, :], c2[None, :]], 0))
    sg = (d["gg"] * (1.0 / np.sqrt(d["vg"] + f(EPS)))).astype(f)
    c2g = (d["bg"] - sg * d["mg"]).astype(f)
    wsg = (d["Wg"].T * sg[None, :]).astype(f)
    shared["wga"] = np.ascontiguousarray(wsg[0:128])
    shared["wgb"] = np.ascontiguousarray(
        np.concatenate([wsg[128:224], c2g[None, :]], 0))
    shared["wv1"] = np.ascontiguousarray(
        d["Wv1"].T.reshape(2, 128, 512).transpose(1, 0, 2))
    shared["bv1"] = np.ascontiguousarray(d["bv1"].reshape(4, 128).T)
    shared["wv2"] = np.ascontiguousarray(
        d["Wv2"].T.reshape(4, 128, 192).transpose(1, 0, 2))
    shared["bv2"] = np.ascontiguousarray(d["bv2"].reshape(2, 96).T)
    shared["wq1"] = np.ascontiguousarray(
        d["Wq1"].T.reshape(2, 128, 64).transpose(1, 0, 2))
    shared["bq1"] = np.ascontiguousarray(d["bq1"].reshape(64, 1))
    shared["wq2"] = np.ascontiguousarray(d["Wq2"].T)
    shared["bq2"] = np.ascontiguousarray(d["bq2"].reshape(1, 1))

    in_maps = []
    x = d["x"]
    for core in range(n_cores):
        smp, h = core // 2, core % 2
        xs = x[smp, :n_pts]  # [n_pts, D]
        xT = np.ascontiguousarray(xs.T)
        xx = (xs * xs).sum(axis=1).astype(f)
        m = dict(shared)
        m["xt0"] = np.ascontiguousarray(
            np.concatenate([2.0 * xT, -xx[None, :]], 0))
        loc = xT[:, h * half:(h + 1) * half]
        m["al1"] = np.ascontiguousarray(
            np.concatenate([loc, np.ones((1, half), f)], 0))
        in_maps.append(m)
    return in_maps


def assemble(results, n_pts, n_cores):
    f = np.float32
    half = n_pts // 2
    nb = n_cores // 2
    xc = np.zeros((nb, n_pts, 224), f)
    gfeat = np.zeros((nb, 256), f)
    vc = np.zeros((nb, 192), f)
    nvs = np.zeros((nb,), f)
    for smp in range(nb):
        for h in range(2):
            r = results[2 * smp + h]
            xc[smp, h * half:(h + 1) * half] = r["xc"]
        r0 = results[2 * smp]
        gfeat[smp] = r0["gf"].T.ravel()
        vc[smp] = r0["vc"].T.ravel()
        nvs[smp] = r0["nv"][0, 0]
    vertex = vc.reshape(nb, MAXV, 3)
    num_v = np.clip(np.round(nvs * f(MAXV)), 1, MAXV).astype(np.int32)
    return vertex, num_v, nvs, gfeat, xc


_CACHE = {}


def _get_kernel(n_pts, n_cores):
    key = (n_pts, n_cores)
    if key not in _CACHE:
        _CACHE[key] = build_kernel(n_pts, n_cores)
    return _CACHE[key]


def kernel(**inputs):
    n_pts, n_cores = N_FULL, 8
    nc = _get_kernel(n_pts, n_cores)
    in_maps = prep_inputs(inputs, n_pts, n_cores)
    res = run_bass_kernel_spmd(nc, in_maps, list(range(n_cores)))
    return assemble(res.results, n_pts, n_cores)
